# revision 23
# baseline (speedup 1.0000x reference)
"""Trainium2 Bass kernel for DNAS PreBasicBlock (mixed-quantization residual block).

Math:
  out = residual + mixed_qconv2(bn_relu2(mixed_qconv1(bn_relu1(x))))

Key optimizations:
  * relu+clip fold: bn_relu followed by clip(.,0,1) == clip(bn(.),0,1); quantized
    activations A_n = clamp(round(n*bn(x)), 0, n) are small integers, exact in
    fp16.  The clamp itself is two chained Relu's on the Scalar engine, the
    round is one DVE op (+M, -M magic).
  * candidate folding: the 9-way weighted conv sum groups by activation bits:
    out = sum_g conv(A_g, W_g), so only 3 convs per layer instead of 9.
  * A_4 = round(A_8/17) and A_2 = round(A_4/5) hold exactly, so only the 8-bit
    grid is stored; 4/2-bit grids are derived per-superchunk on the fly.
  * both layers conv in a single fp16 pass per group (weights scaled by 2^8 to
    dodge fp16 subnormals; the PSUM->SBUF copy applies 2^-8).  The fp16 weight
    error (~2^-11) costs ~2e-3 rel err through the layer-2 quantizer --
    measured 1.2e-2 absmax total vs the 2e-2 budget.
  * max|tanh(w)| == tanh(max|w|): per-candidate weight normalizers come from a
    reduce over the raw weights plus one tiny tanh; computed per-GROUP so the
    first group's conv passes start as early as possible.
  * weights are DMA'd once: amax reduce and tanh read the same resident tile.
  * emission order tuned for in-order queues: group-2 weight DMAs go out
    before x; BN stats split across DVE (sum) and ACT (sum-of-squares); first
    conv pass starts ~50us in instead of ~115us.
  * conv as 9 shift-matmuls per pass accumulating in PSUM; data-parallel over
    batch across 8 cores; BN batch stats via two tiny (1KB) AllReduces.
"""
import sys

sys.path.insert(0, "/opt/trn_rl_repo")

import numpy as np

import concourse.bass as bass
import concourse.tile as tile
from concourse import bacc, bass_utils, mybir
from concourse.tile_rust import add_dep_helper

dt = mybir.dt
Alu = mybir.AluOpType
Act = mybir.ActivationFunctionType

N_CORES = 8
B, C, H, W = 32, 128, 56, 56
BS = B // N_CORES          # batch shard per core
HP, WP = H + 2, W + 2      # padded image: 1 row/col of zeros on each side
IMG = HP * WP              # 3364
APIX = BS * IMG            # 13456
BASE = WP + 1              # first valid flat offset within an image: 59
VSTART = BASE
VEND = (BS - 1) * IMG + H * WP + W + 1  # one past last valid: 13397
CHUNK = 512
NCHUNK = -(-(VEND - VSTART) // CHUNK)  # 27
SCCH = 3                   # chunks per superchunk
SLEN = SCCH * CHUNK        # 1536
NSC = -(-NCHUNK // SCCH)   # 9
STG = 1664                 # derive-staging width (halo + alignment slack)
NPIX_IMG = H * W           # 3136
IMG_SPAN = H * WP          # 3248: rows 1..56 as 56 x 58 view
NTOT = float(B * H * W)    # BN divisor 100352
MAGIC = 12582912.0         # 1.5*2^23: fp32 round-to-int via add/sub
MAGICH = 1536.0            # 1.5*2^10: fp16 round-to-int via f32->fp16 convert
EPS = 1e-5
WSCALE = 256.0             # fp16 weight scale (off subnormals)

BITS = [2, 4, 8]
NW = [2 ** BITS[k // 3] - 1 for k in range(9)]   # weight levels per candidate
NA = [2 ** BITS[k % 3] - 1 for k in range(9)]    # activation levels per candidate
GROUP_KS = {2: [2, 5, 8], 1: [1, 4, 7], 0: [0, 3, 6]}
GORDER = [2, 1, 0]         # ba=8 group first
TAPS = [(ky - 1) * WP + (kx - 1) for ky in range(3) for kx in range(3)]

# which superchunks become available after which image is quantized:
SC_NEEDS_IMG = []
for _s in range(NSC):
    _hi = min(VSTART + 1536 * (_s + 1) + 59, APIX)
    SC_NEEDS_IMG.append(min((_hi - 1) // IMG, BS - 1))

# image i's last valid pixel lives in superchunk:
IMG_LAST_SC = {}
for _i in range(BS):
    _last = _i * IMG + H * WP + W
    _s = min((_last - VSTART) // SLEN, NSC - 1)
    IMG_LAST_SC[_s] = _i

_CACHE = {}


def _chunks_of_sc(s):
    out = []
    for c in range(SCCH):
        ci = s * SCCH + c
        if ci >= NCHUNK:
            break
        gs = VSTART + ci * CHUNK
        ln = min(CHUNK, VEND - gs)
        out.append((c * CHUNK, gs, ln))
    return out


def _build():
    nc = bacc.Bacc("TRN2", target_bir_lowering=False, debug=False,
                   num_devices=N_CORES)

    x_in = nc.dram_tensor("x", [BS, C, H, W], dt.float32, kind="ExternalInput")
    w1_in = nc.dram_tensor("conv1_w", [9, C, C, 3, 3], dt.float32, kind="ExternalInput")
    w2_in = nc.dram_tensor("conv2_w", [9, C, C, 3, 3], dt.float32, kind="ExternalInput")
    g1_in = nc.dram_tensor("gamma1", [C], dt.float32, kind="ExternalInput")
    b1_in = nc.dram_tensor("beta1", [C], dt.float32, kind="ExternalInput")
    g2_in = nc.dram_tensor("gamma2", [C], dt.float32, kind="ExternalInput")
    b2_in = nc.dram_tensor("beta2", [C], dt.float32, kind="ExternalInput")
    p1_in = nc.dram_tensor("p1", [9], dt.float32, kind="ExternalInput")
    p2_in = nc.dram_tensor("p2", [9], dt.float32, kind="ExternalInput")
    gn1_in = nc.dram_tensor("gn1", [9], dt.float32, kind="ExternalInput")
    gn2_in = nc.dram_tensor("gn2", [9], dt.float32, kind="ExternalInput")
    tau_in = nc.dram_tensor("tau", [1], dt.float32, kind="ExternalInput")
    consts_in = nc.dram_tensor("consts", [1, 27], dt.float32, kind="ExternalInput")
    out_dram = nc.dram_tensor("out", [BS, C, H, W], dt.float32, kind="ExternalOutput")

    from concourse.masks import make_identity

    with tile.TileContext(nc) as tc:
        with tc.tile_pool(name="main", bufs=1) as sb, \
             tc.tile_pool(name="ps", bufs=1, space="PSUM") as ps, \
             tc.tile_pool(name="dram", bufs=1, space="DRAM") as dram:

            # ---------- static tiles / input DMAs ----------
            ident = sb.tile([128, 128], dt.float32)
            make_identity(nc, ident[:])

            # memset on DVE, NOT GpSimd: the first gpsimd-queue instruction
            # gates the collectives firmware's one-time ~38us bootstrap
            # barrier, which in turn gates the BN1 AllReduce start.
            A8 = sb.tile([C, APIX], dt.float16, tag="A8")
            nc.vector.memset(A8[:], 0.0)  # zero borders once; writes stay interior

            wsrc1 = w1_in.ap().rearrange("k o i a b -> k o (i a b)")
            wsrc2 = w2_in.ap().rearrange("k o i a b -> k o (i a b)")

            def wdma(wsrc, ks, tag, dep=None):
                raws = {}
                for k in ks:
                    raw = sb.tile([C, 1152], dt.float32, tag="wraw", bufs=6,
                                  name=f"raw_{tag}_{k}")
                    d = nc.sync.dma_start(raw[:], wsrc[k])
                    if dep is not None:
                        add_dep_helper(d.ins, dep, reason="weight DMA yields to x")
                    raws[k] = raw
                return raws

            # x goes out first -- the BN1 AllReduce (gated on all of x) is the
            # startup critical path; weights have ~20us of slack behind it.
            x_sb = sb.tile([C, BS * NPIX_IMG], dt.float32, tag="big", name="x_sb")
            x_src = x_in.ap().rearrange("b c h w -> c b (h w)")
            x_dmas = []
            for i in range(BS):
                for hh in range(2):  # halves spread wider across DMA queues
                    lo_ = (i * 2 + hh) * (NPIX_IMG // 2)
                    x_dmas.append(nc.sync.dma_start(
                        x_sb[:, lo_:lo_ + NPIX_IMG // 2],
                        x_src[:, i, hh * (NPIX_IMG // 2):(hh + 1) * (NPIX_IMG // 2)]))

            raws1 = wdma(wsrc1, GROUP_KS[2], "w1")

            def row(name, t, n):
                r = sb.tile([1, n], dt.float32, name=name)
                nc.sync.dma_start(r[:], t.ap()[None, :])
                return r

            p1r = row("p1r", p1_in, 9)
            gn1r = row("gn1r", gn1_in, 9)
            p2r = row("p2r", p2_in, 9)
            gn2r = row("gn2r", gn2_in, 9)
            taur = row("taur", tau_in, 1)
            constsr = sb.tile([1, 27], dt.float32)
            nc.sync.dma_start(constsr[:], consts_in.ap())

            def col128(name, t):
                r = sb.tile([C, 1], dt.float32, name=name)
                nc.sync.dma_start(r[:], t.ap()[:, None])
                return r

            gam1, bet1 = col128("gam1", g1_in), col128("bet1", b1_in)
            gam2, bet2 = col128("gam2", g2_in), col128("bet2", b2_in)

            rtau = sb.tile([1, 1], dt.float32)
            nc.vector.reciprocal(rtau[:], taur[:])

            # ---------- per-layer softmax -> alpha/gamma strip -> broadcast ----------
            def softmax_strip(pr, gnr, tag):
                u = sb.tile([1, 9], dt.float32, name=f"u_{tag}")
                nc.vector.tensor_tensor(u[:], pr[:], gnr[:], Alu.add)
                nc.vector.tensor_scalar(u[:], u[:], rtau[:, 0:1], None, Alu.mult)
                mx = sb.tile([1, 1], dt.float32, name=f"mx_{tag}")
                nc.vector.tensor_reduce(mx[:], u[:], axis=mybir.AxisListType.X,
                                        op=Alu.max)
                nmx = sb.tile([1, 1], dt.float32, name=f"nmx_{tag}")
                nc.vector.tensor_scalar(nmx[:], mx[:], -1.0, None, Alu.mult)
                e = sb.tile([1, 9], dt.float32, name=f"e_{tag}")
                nc.scalar.activation(e[:], u[:], Act.Exp, bias=nmx[:, 0:1], scale=1.0)
                ssum = sb.tile([1, 1], dt.float32, name=f"ss_{tag}")
                nc.vector.tensor_reduce(ssum[:], e[:], axis=mybir.AxisListType.X,
                                        op=Alu.add)
                rsum = sb.tile([1, 1], dt.float32, name=f"rs_{tag}")
                nc.vector.reciprocal(rsum[:], ssum[:])
                wrow = sb.tile([1, 9], dt.float32, name=f"w_{tag}")
                nc.vector.tensor_scalar(wrow[:], e[:], rsum[:, 0:1], None, Alu.mult)
                strip = sb.tile([1, 12], dt.float32, name=f"strip_{tag}")
                nc.vector.tensor_tensor(strip[:, 0:9], wrow[:], constsr[:, 0:9],
                                        Alu.mult)
                pe1 = sb.tile([1, 9], dt.float32, name=f"pe1_{tag}")
                nc.vector.tensor_tensor(pe1[:], wrow[:], constsr[:, 9:18], Alu.mult)
                pe13 = pe1[:].rearrange("p (i g) -> p i g", g=3)
                for g in range(3):
                    nc.vector.tensor_reduce(strip[:, 9 + g:10 + g], pe13[:, :, g],
                                            axis=mybir.AxisListType.X, op=Alu.add,
                                            negate=True)
                bcast = sb.tile([C, 12], dt.float32, name=f"bcast_{tag}")
                nc.gpsimd.partition_broadcast(bcast[:], strip[:])
                return bcast

            # ---------- BN batch stats: sum on DVE, sum-of-squares on ACT ----
            def img_stats_flat(src2d, ssum, ssq, col, tag):
                nc.vector.tensor_reduce(ssum[:, col:col + 1], src2d,
                                        axis=mybir.AxisListType.X, op=Alu.add)
                n = src2d.shape[1]
                scr2 = sb.tile([C, NPIX_IMG], dt.float32, tag="scr", bufs=2,
                               name=f"scq_{tag}_{col}")
                nc.scalar.activation(scr2[:, 0:n], src2d, Act.Square, bias=0.0,
                                     scale=1.0, accum_out=ssq[:, col:col + 1])

            def img_stats_3d(src3d, ssum, ssq, col, rowscr, tag):
                # strided source: DVE reduces innermost dim then rows
                a = src3d.shape[1]
                nc.vector.tensor_reduce(rowscr[:, 0:a], src3d,
                                        axis=mybir.AxisListType.X, op=Alu.add)
                nc.vector.tensor_reduce(ssum[:, col:col + 1], rowscr[:, 0:a],
                                        axis=mybir.AxisListType.X, op=Alu.add)
                n = a * src3d.shape[2]
                scr2 = sb.tile([C, NPIX_IMG], dt.float32, tag="scr", bufs=2,
                               name=f"scq_{tag}_{col}")
                scr23 = scr2[:, 0:n].rearrange("p (a b) -> p a b",
                                               b=src3d.shape[2])
                nc.scalar.activation(scr23, src3d, Act.Square, bias=0.0,
                                     scale=1.0, accum_out=ssq[:, col:col + 1])

            def ar_local(ssum, ssq, tag):
                loc = sb.tile([C, 2], dt.float32, name=f"loc_{tag}")
                nc.vector.tensor_reduce(loc[:, 0:1], ssum[:],
                                        axis=mybir.AxisListType.X, op=Alu.add)
                nc.vector.tensor_reduce(loc[:, 1:2], ssq[:],
                                        axis=mybir.AxisListType.X, op=Alu.add)
                cin = dram.tile([C, 2], dt.float32, name=f"ccin_{tag}")
                nc.sync.dma_start(cin[:], loc[:])
                return cin

            def ar_go(cin, tag):
                cout = dram.tile([C, 2], dt.float32, addr_space="Shared",
                                 name=f"ccout_{tag}")
                nc.gpsimd.collective_compute(
                    "AllReduce", Alu.add,
                    replica_groups=[list(range(N_CORES))],
                    ins=[cin.opt()], outs=[cout.opt()])
                glob = sb.tile([C, 2], dt.float32, name=f"glob_{tag}")
                nc.sync.dma_start(glob[:], cout[:])
                return glob

            def bn_scalars(glob, gam, bet, tag):
                def t1(name):
                    return sb.tile([C, 1], dt.float32, name=f"{name}_{tag}")
                mean, e2, msq, var, ve = (t1("mean"), t1("e2"), t1("msq"),
                                          t1("var"), t1("ve"))
                nc.vector.tensor_scalar(mean[:], glob[:, 0:1], 1.0 / NTOT, None,
                                        Alu.mult)
                nc.vector.tensor_scalar(e2[:], glob[:, 1:2], 1.0 / NTOT, None,
                                        Alu.mult)
                nc.vector.tensor_tensor(msq[:], mean[:], mean[:], Alu.mult)
                nc.vector.tensor_tensor(var[:], e2[:], msq[:], Alu.subtract)
                nc.vector.tensor_scalar(ve[:], var[:], EPS, None, Alu.add)
                sq, y = t1("sq"), t1("y0")
                nc.scalar.activation(sq[:], ve[:], Act.Sqrt, bias=0.0, scale=1.0)
                nc.vector.reciprocal(y[:], sq[:])
                for it in range(2):  # Newton: y <- y*(1.5 - 0.5*ve*y^2)
                    tt1, tt2, tt3, yn = (t1(f"n{it}a"), t1(f"n{it}b"),
                                         t1(f"n{it}c"), t1(f"y{it + 1}"))
                    nc.vector.tensor_tensor(tt1[:], y[:], y[:], Alu.mult)
                    nc.vector.tensor_tensor(tt2[:], tt1[:], ve[:], Alu.mult)
                    nc.vector.tensor_scalar(tt3[:], tt2[:], -0.5, 1.5, Alu.mult,
                                            Alu.add)
                    nc.vector.tensor_tensor(yn[:], y[:], tt3[:], Alu.mult)
                    y = yn
                sbn, bt, sq_, bq_ = (t1("sbn"), t1("bt"), t1("sclq"), t1("biasq"))
                nc.vector.tensor_tensor(sbn[:], gam[:], y[:], Alu.mult)
                nc.vector.tensor_tensor(bt[:], mean[:], sbn[:], Alu.mult)
                # u = 255*(s*x + b): sq = 255*s ; bq = 255*(beta - mean*s)
                nc.vector.tensor_scalar(bq_[:], bt[:], -255.0, bet255(bet, tag),
                                        Alu.mult, Alu.add)
                nc.vector.tensor_scalar(sq_[:], sbn[:], 255.0, None, Alu.mult)
                return sq_, bq_

            _bet255 = {}

            def bet255(bet, tag):
                if tag not in _bet255:
                    b = sb.tile([C, 1], dt.float32, name=f"bet255_{tag}")
                    nc.vector.tensor_scalar(b[:], bet[:], 255.0, None, Alu.mult)
                    _bet255[tag] = b
                return _bet255[tag][:, 0:1]

            # ---------- quantize one image into the 8-bit grid ----------
            def quantize_img(src3d, A8t, sq_, bq_, i, tag, r0=0, r1=H):
                # u = relu(255*(s*x+b)); round+clamp: (min(u,255)+M)-M
                nr = r1 - r0
                u = sb.tile([C, NPIX_IMG], dt.float32, tag="scr", bufs=2,
                            name=f"qu_{tag}_{i}_{r0}")
                u3 = u[:, 0:nr * W].rearrange("p (a b) -> p a b", a=nr)
                nc.scalar.activation(u3, src3d[:, r0:r1], Act.Relu,
                                     bias=bq_[:, 0:1], scale=sq_[:, 0:1])
                nc.vector.tensor_scalar(u[:, 0:nr * W], u[:, 0:nr * W], 255.0,
                                        MAGIC, Alu.min, Alu.add)
                dst = A8t[:, i * IMG + BASE + r0 * WP:
                          i * IMG + BASE + r1 * WP]
                dst3 = dst.rearrange("p (a b) -> p a b", b=WP)[:, :, 0:W]
                nc.vector.tensor_scalar(dst3, u3, MAGIC, None, Alu.subtract)

            # ---------- weight preparation ----------
            def group_r2(raws, ks, gi, tag):
                """abs-max per candidate -> r2 = 1/(2 tanh(amax)) column [C,3].

                abs-max per partition on DVE (the GpSimd cross-lane reduce
                ignores apply_absolute_value on hardware), then the partition
                max of the nonnegative column on GpSimd -- no transpose."""
                amaxc = sb.tile([C, 3], dt.float32, name=f"amaxc_{tag}_{gi}")
                mrow = sb.tile([1, 3], dt.float32, name=f"mrow_{tag}_{gi}")
                for j, k in enumerate(ks):
                    nc.vector.tensor_reduce(amaxc[:, j:j + 1], raws[k][:],
                                            axis=mybir.AxisListType.X,
                                            op=Alu.max,
                                            apply_absolute_value=True)
                    nc.gpsimd.tensor_reduce(mrow[0:1, j:j + 1],
                                            amaxc[:, j:j + 1],
                                            axis=mybir.AxisListType.C,
                                            op=Alu.max)
                tam = sb.tile([1, 3], dt.float32, name=f"tam_{tag}_{gi}")
                nc.scalar.activation(tam[:], mrow[:], Act.Tanh, bias=0.0, scale=1.0)
                a2 = sb.tile([1, 3], dt.float32, name=f"a2_{tag}_{gi}")
                nc.vector.tensor_scalar(a2[:], tam[:], 2.0, None, Alu.mult)
                r2r = sb.tile([1, 3], dt.float32, name=f"r2r_{tag}_{gi}")
                nc.vector.reciprocal(r2r[:], a2[:])
                r2g = sb.tile([C, 3], dt.float32, name=f"r2g_{tag}_{gi}")
                nc.gpsimd.partition_broadcast(r2g[:], r2r[:])
                return r2g

            def prep_chain(raws, ks, r2g, bcast, tag, g):
                """accumulate one ba-group's quantized candidates -> wacc."""
                wacc = None
                for j, k in enumerate(ks):
                    th = sb.tile([C, 1152], dt.float32, tag="wth", bufs=2,
                                 name=f"th_{tag}_{k}")
                    nc.scalar.activation(th[:], raws[k][:], Act.Tanh, bias=0.0,
                                         scale=1.0)
                    # wn = th/(2amax)+0.5 ; u2 = wn*nw + M (rounds) ; m = u2-M
                    # (+0.5*nw must NOT fold into M: M+127.5 isn't fp32-exact)
                    nc.vector.tensor_scalar(th[:], th[:], r2g[:, j:j + 1], 0.5,
                                            Alu.mult, Alu.add)
                    nc.vector.tensor_scalar(th[:], th[:], float(NW[k]), MAGIC,
                                            Alu.mult, Alu.add)
                    nc.vector.tensor_scalar(th[:], th[:], MAGIC, None,
                                            Alu.subtract)
                    if j == 0:
                        wacc = sb.tile([C, 1152], dt.float32, tag="wacc",
                                       bufs=2, name=f"wacc_{tag}_{g}_{j}")
                        nc.vector.tensor_scalar(wacc[:], th[:], bcast[:, k:k + 1],
                                                bcast[:, 9 + g:10 + g],
                                                Alu.mult, Alu.add)
                    else:
                        nxt = sb.tile([C, 1152], dt.float32, tag="wacc",
                                      bufs=2, name=f"wacc_{tag}_{g}_{j}")
                        nc.vector.scalar_tensor_tensor(nxt[:], th[:],
                                                       bcast[:, k:k + 1], wacc[:],
                                                       Alu.mult, Alu.add)
                        wacc = nxt
                return wacc

            def prep_transpose(wacc, tag, g, Wt):
                w3 = wacc[:].rearrange("p (i t) -> p i t", t=9)
                for t in range(9):
                    tp = ps.tile([128, 128], dt.float32, tag="tps", bufs=2,
                                 name=f"tp_{tag}_{g}_{t}")
                    nc.tensor.transpose(tp[:], w3[:, :, t], ident[:])
                    nc.scalar.activation(Wt[g][:, t, :], tp[:], Act.Copy,
                                         bias=0.0, scale=WSCALE)

            def alloc_W(tag):
                return [sb.tile([C, 9, C], dt.float16, name=f"W_{tag}_{g}")
                        for g in range(3)]

            # ---------- conv pieces ----------
            def derive_sc(src8, s, tag):
                start = VSTART + s * SLEN
                lo = min(max((start - 64) & ~1, 0), APIX - STG)
                a4s = sb.tile([C, STG], dt.float16, tag="a4s", bufs=3,
                              name=f"a4_{tag}_{s}")
                nc.vector.tensor_scalar(a4s[:], src8[:, lo:lo + STG], 1.0 / 17.0,
                                        MAGICH, Alu.mult, Alu.add)
                nc.vector.tensor_scalar(a4s[:], a4s[:], MAGICH, None, Alu.subtract)
                a2s = sb.tile([C, STG], dt.float16, tag="a2s", bufs=3,
                              name=f"a2_{tag}_{s}")
                nc.vector.tensor_scalar(a2s[:], a4s[:], 1.0 / 5.0, MAGICH,
                                        Alu.mult, Alu.add)
                nc.vector.tensor_scalar(a2s[:], a2s[:], MAGICH, None, Alu.subtract)
                return a4s, a2s, lo

            def conv_sc(Wt, src8, a4s, a2s, lo, cdst, s, tag, groups=None):
                """emit conv passes for superchunk s; groups=None -> all."""
                chunks = _chunks_of_sc(s)
                start = VSTART + s * SLEN
                pt = _sc_psum(tag, s)
                for pi, g in enumerate(GORDER):
                    if groups is not None and g not in groups:
                        continue
                    for t in range(9):
                        off = TAPS[t]
                        for (pcol, gs, ln) in chunks:
                            if g == 2:
                                rhs = src8[:, gs + off:gs + off + ln]
                            elif g == 1:
                                rhs = a4s[:, gs + off - lo:gs + off - lo + ln]
                            else:
                                rhs = a2s[:, gs + off - lo:gs + off - lo + ln]
                            nc.tensor.matmul(
                                pt[:, pcol:pcol + ln], Wt[g][:, t, :], rhs,
                                start=(pi == 0 and t == 0),
                                stop=(pi == 2 and t == 8))
                if groups is None or 0 in groups:
                    sc_end = min(start + SLEN, VEND)
                    nc.scalar.activation(cdst[:, start:sc_end],
                                         pt[:, 0:sc_end - start], Act.Copy,
                                         bias=0.0, scale=1.0 / WSCALE)

            _psums = {}

            def _sc_psum(tag, s):
                key = (tag, s)
                if key not in _psums:
                    _psums[key] = ps.tile([128, SLEN], dt.float32, tag="cps",
                                          bufs=2, name=f"ps_{tag}_{s}")
                return _psums[key]

            # ================= LAYER 1 =================
            ssum1 = sb.tile([C, 5], dt.float32)
            ssq1 = sb.tile([C, 5], dt.float32)
            nc.vector.memset(ssum1[:], 0.0)
            nc.vector.memset(ssq1[:], 0.0)
            x3 = x_sb[:].rearrange("p (b a w) -> p b a w", b=BS, a=H)
            for i in range(BS):
                img_stats_flat(x_sb[:, i * NPIX_IMG:(i + 1) * NPIX_IMG],
                               ssum1, ssq1, i, "s1")
            cin1 = ar_local(ssum1, ssq1, "c1")

            # g1 candidates land in fresh slots; g0's DMAs are emitted only
            # after g2's chain reads (slot reuse: 6 bufs, 9 candidates)
            raws1.update(wdma(wsrc1, GROUP_KS[1], "w1"))

            r2g1 = {2: group_r2(raws1, GROUP_KS[2], 0, "w1")}
            r2g1[1] = group_r2(raws1, GROUP_KS[1], 1, "w1")

            glob1 = ar_go(cin1, "c1")
            bc1 = softmax_strip(p1r, gn1r, "l1")
            bc2 = softmax_strip(p2r, gn2r, "l2")

            W1 = alloc_W("w1")
            c1 = sb.tile([C, APIX], dt.float32, tag="big", name="c1buf")
            ssum2 = sb.tile([C, 5], dt.float32)
            ssq2 = sb.tile([C, 5], dt.float32)
            nc.vector.memset(ssum2[:], 0.0)
            nc.vector.memset(ssq2[:], 0.0)
            rowscr = sb.tile([C, H], dt.float32, name="rowscr")

            # all three groups' weights finalize while the collectives
            # bootstrap barrier + AllReduce (~83us) are in flight
            wacc = prep_chain(raws1, GROUP_KS[2], r2g1[2], bc1, "w1", 2)
            prep_transpose(wacc, "w1", 2, W1)
            raws1.update(wdma(wsrc1, GROUP_KS[0], "w1"))
            wacc = prep_chain(raws1, GROUP_KS[1], r2g1[1], bc1, "w1", 1)
            prep_transpose(wacc, "w1", 1, W1)
            r2g1[0] = group_r2(raws1, GROUP_KS[0], 2, "w1")
            wacc = prep_chain(raws1, GROUP_KS[0], r2g1[0], bc1, "w1", 0)
            prep_transpose(wacc, "w1", 0, W1)

            nsq1, nbq1 = bn_scalars(glob1, gam1, bet1, "bn1")

            def c1_img3d(i, r0=0, r1=H):
                off = i * IMG + BASE + r0 * WP
                v = c1[:, off:off + (r1 - r0) * WP]
                return v.rearrange("p (a b) -> p a b", b=WP)[:, :, 0:W]

            # NOTE: all of x must be consumed (quantized) before conv1's first
            # PSUM copy writes c1 -- they share one SBUF slot and the slot
            # handover is tile-granular.  The first copy comes after group-0's
            # sc0 pass, so imgs 2/3 quantize between the g1 and g0 passes.
            # The g2 passes read only the A8 grid, so they start as soon as
            # img0 is quantized; derives and later-group weight chains overlap
            # the g2 matmul sweep.
            # rows 0..29 of img0 cover everything superchunk 0 reads, so the
            # first conv pass starts one half-image-quantize after BN1 lands
            quantize_img(x3[:, 0], A8, nsq1, nbq1, 0, "q1", 0, 30)
            a4s0, a2s0, lo0 = derive_sc(A8[:], 0, "cv1")
            conv_sc(W1, A8[:], a4s0, a2s0, lo0, c1, 0, "cv1", groups=[2])
            quantize_img(x3[:, 0], A8, nsq1, nbq1, 0, "q1", 30, H)
            quantize_img(x3[:, 1], A8, nsq1, nbq1, 1, "q1")
            a4s1, a2s1, lo1 = derive_sc(A8[:], 1, "cv1")
            conv_sc(W1, A8[:], a4s1, a2s1, lo1, c1, 1, "cv1", groups=[2])

            conv_sc(W1, A8[:], a4s0, a2s0, lo0, c1, 0, "cv1", groups=[1])
            conv_sc(W1, A8[:], a4s1, a2s1, lo1, c1, 1, "cv1", groups=[1])

            quantize_img(x3[:, 2], A8, nsq1, nbq1, 2, "q1")
            quantize_img(x3[:, 3], A8, nsq1, nbq1, 3, "q1")

            conv_sc(W1, A8[:], a4s0, a2s0, lo0, c1, 0, "cv1", groups=[0])
            conv_sc(W1, A8[:], a4s1, a2s1, lo1, c1, 1, "cv1", groups=[0])

            def cv1_after_sc(s):
                if s == 7:  # partial img3 stats (rows 1..37 available)
                    img_stats_3d(c1_img3d(3, 0, 37), ssum2, ssq2, 3, rowscr, "s2")
                if s in IMG_LAST_SC:
                    i = IMG_LAST_SC[s]
                    if i == 3:
                        img_stats_3d(c1_img3d(3, 37, H), ssum2, ssq2, 4, rowscr,
                                     "s2")
                    else:
                        img_stats_3d(c1_img3d(i), ssum2, ssq2, i, rowscr, "s2")

            cv1_after_sc(0), cv1_after_sc(1)
            r2g2 = {}
            for s in range(2, NSC):
                a4s, a2s, lo = derive_sc(A8[:], s, "cv1")
                conv_sc(W1, A8[:], a4s, a2s, lo, c1, s, "cv1")
                cv1_after_sc(s)
                # layer-2 weight prep spread across conv1 superchunks so each
                # group's PE transposes are ready when the FIFO reaches them
                if s == 2:
                    raws2 = wdma(wsrc2, GROUP_KS[2] + GROUP_KS[1], "w2")
                    W2 = alloc_W("w2")
                elif s in (3, 4, 5):
                    g = GORDER[s - 3]
                    r2g2[g] = group_r2(raws2, GROUP_KS[g], s - 3, "w2")
                    wacc_g = prep_chain(raws2, GROUP_KS[g], r2g2[g], bc2,
                                        "w2", g)
                    prep_transpose(wacc_g, "w2", g, W2)
                    if s == 4:
                        # g0 slots reuse g2's: DMA only after g2's chain reads
                        raws2.update(wdma(wsrc2, GROUP_KS[0], "w2"))

            # ================= LAYER 2 =================
            cin2 = ar_local(ssum2, ssq2, "c2")
            glob2 = ar_go(cin2, "c2")
            nsq2, nbq2 = bn_scalars(glob2, gam2, bet2, "bn2")

            # layer-2 grid shares the A8 slot: zero borders are bit-identical
            A8h = sb.tile([C, APIX], dt.float16, tag="A8", name="A8h")

            out_v = out_dram.ap().rearrange("b c h w -> c b (h w)")

            def residual_out(i, r0=0, r1=H, pre=None):
                n = (r1 - r0) * W
                if pre is None:
                    xr = sb.tile([C, NPIX_IMG], dt.float32, tag="scr", bufs=2,
                                 name=f"xr_{i}_{r0}")
                    nc.sync.dma_start(xr[:, 0:n], x_src[:, i, r0 * W:r1 * W])
                    xrv = xr[:, 0:n]
                else:
                    xrv = pre[:]
                xr3 = xrv.rearrange("p (a b) -> p a b", a=r1 - r0)
                nc.vector.tensor_tensor(xr3, xr3, c1_img3d(i, r0, r1), Alu.add)
                nc.sync.dma_start(out_v[:, i, r0 * W:r1 * W], xrv)

            # img3's residual x is prefetched into dedicated tiles so the
            # final superchunk's add + store isn't DMA-latency-bound
            xr3a = sb.tile([C, 37 * W], dt.float32, name="xr3a")
            xr3b = sb.tile([C, (H - 37) * W], dt.float32, name="xr3b")

            _q2_done = {}
            for s in range(0, NSC):
                need = SC_NEEDS_IMG[s]
                for i in range(BS):
                    if need >= i and not _q2_done.get(i):
                        quantize_img(c1_img3d(i), A8h, nsq2, nbq2, i, "q2")
                        _q2_done[i] = True
                a4s, a2s, lo = derive_sc(A8h[:], s, "cv2")
                conv_sc(W2, A8h[:], a4s, a2s, lo, c1, s, "cv2")
                if s == 5:
                    nc.sync.dma_start(xr3a[:], x_src[:, 3, 0:37 * W])
                    nc.sync.dma_start(xr3b[:], x_src[:, 3, 37 * W:H * W])
                if s == 7:
                    residual_out(3, 0, 37, pre=xr3a)
                if s in IMG_LAST_SC:
                    i = IMG_LAST_SC[s]
                    if i == 3:
                        residual_out(3, 37, H, pre=xr3b)
                    else:
                        residual_out(i)

    nc.compile()
    return nc


def _consts():
    c = np.zeros((1, 27), np.float32)
    for k in range(9):
        c[0, k] = 2.0 / (NW[k] * NA[k])
        c[0, 9 + k] = 1.0 / NA[k]
        c[0, 18 + k] = float(NW[k])
    return c


def _in_maps(inputs):
    x = np.ascontiguousarray(inputs["x"], dtype=np.float32)
    shared = {
        "conv1_w": np.ascontiguousarray(inputs["conv1_w"], dtype=np.float32),
        "conv2_w": np.ascontiguousarray(inputs["conv2_w"], dtype=np.float32),
        "gamma1": np.ascontiguousarray(inputs["gamma1"], dtype=np.float32),
        "beta1": np.ascontiguousarray(inputs["beta1"], dtype=np.float32),
        "gamma2": np.ascontiguousarray(inputs["gamma2"], dtype=np.float32),
        "beta2": np.ascontiguousarray(inputs["beta2"], dtype=np.float32),
        "p1": np.ascontiguousarray(inputs["p1"], dtype=np.float32),
        "p2": np.ascontiguousarray(inputs["p2"], dtype=np.float32),
        "gn1": np.ascontiguousarray(inputs["gn1"], dtype=np.float32),
        "gn2": np.ascontiguousarray(inputs["gn2"], dtype=np.float32),
        "tau": np.asarray(inputs["tau"], dtype=np.float32).reshape(1),
        "consts": _consts(),
    }
    return [dict(shared, x=x[c * BS:(c + 1) * BS]) for c in range(N_CORES)]


def _get_nc():
    if "nc" not in _CACHE:
        _CACHE["nc"] = _build()
    return _CACHE["nc"]


def _run(in_maps, trace=False):
    nc = _get_nc()
    return bass_utils.run_bass_kernel_spmd(
        nc, in_maps, core_ids=list(range(N_CORES)), trace=trace)


def kernel(**inputs) -> np.ndarray:
    res = _run(_in_maps(inputs))
    return np.concatenate([res.results[c]["out"] for c in range(N_CORES)], axis=0)


# revision 26
# speedup vs baseline: 1.0636x; 1.0636x over previous
"""Trainium2 Bass kernel for DNAS PreBasicBlock (mixed-quantization residual block).

Math:
  out = residual + mixed_qconv2(bn_relu2(mixed_qconv1(bn_relu1(x))))

Key optimizations:
  * relu+clip fold: bn_relu followed by clip(.,0,1) == clip(bn(.),0,1); quantized
    activations A_n = clamp(round(n*bn(x)), 0, n) are small integers, exact in
    fp16.  The clamp itself is two chained Relu's on the Scalar engine, the
    round is one DVE op (+M, -M magic).
  * candidate folding: the 9-way weighted conv sum groups by activation bits:
    out = sum_g conv(A_g, W_g), so only 3 convs per layer instead of 9.
  * A_4 = round(A_8/17) and A_2 = round(A_4/5) hold exactly, so only the 8-bit
    grid is stored; 4/2-bit grids are derived per-superchunk on the fly.
  * both layers conv in a single fp16 pass per group (weights scaled by 2^8 to
    dodge fp16 subnormals; the PSUM->SBUF copy applies 2^-8).  The fp16 weight
    error (~2^-11) costs ~2e-3 rel err through the layer-2 quantizer --
    measured 1.2e-2 absmax total vs the 2e-2 budget.
  * max|tanh(w)| == tanh(max|w|): per-candidate weight normalizers come from a
    reduce over the raw weights plus one tiny tanh; computed per-GROUP so the
    first group's conv passes start as early as possible.
  * weights are DMA'd once: amax reduce and tanh read the same resident tile.
  * emission order tuned for in-order queues: group-2 weight DMAs go out
    before x; BN stats split across DVE (sum) and ACT (sum-of-squares); first
    conv pass starts ~50us in instead of ~115us.
  * conv as 9 shift-matmuls per pass accumulating in PSUM; data-parallel over
    batch across 8 cores; BN batch stats via two tiny (1KB) AllReduces.
"""
import sys

sys.path.insert(0, "/opt/trn_rl_repo")

import numpy as np

import concourse.bass as bass
import concourse.tile as tile
from concourse import bacc, bass_utils, mybir
from concourse.tile_rust import add_dep_helper

dt = mybir.dt
Alu = mybir.AluOpType
Act = mybir.ActivationFunctionType

N_CORES = 8
B, C, H, W = 32, 128, 56, 56
BS = B // N_CORES          # batch shard per core
HP, WP = H + 2, W + 2      # padded image: 1 row/col of zeros on each side
IMG = HP * WP              # 3364
APIX = BS * IMG            # 13456
BASE = WP + 1              # first valid flat offset within an image: 59
VSTART = BASE
VEND = (BS - 1) * IMG + H * WP + W + 1  # one past last valid: 13397
CHUNK = 512
NCHUNK = -(-(VEND - VSTART) // CHUNK)  # 27
SCCH = 3                   # chunks per superchunk
SLEN = SCCH * CHUNK        # 1536
NSC = -(-NCHUNK // SCCH)   # 9
STG = 1664                 # derive-staging width (halo + alignment slack)
NPIX_IMG = H * W           # 3136
IMG_SPAN = H * WP          # 3248: rows 1..56 as 56 x 58 view
NTOT = float(B * H * W)    # BN divisor 100352
MAGIC = 12582912.0         # 1.5*2^23: fp32 round-to-int via add/sub
MAGICH = 1536.0            # 1.5*2^10: fp16 round-to-int via f32->fp16 convert
EPS = 1e-5
WSCALE = 256.0             # fp16 weight scale (off subnormals)

BITS = [2, 4, 8]
NW = [2 ** BITS[k // 3] - 1 for k in range(9)]   # weight levels per candidate
NA = [2 ** BITS[k % 3] - 1 for k in range(9)]    # activation levels per candidate
GROUP_KS = {2: [2, 5, 8], 1: [1, 4, 7], 0: [0, 3, 6]}
GORDER = [2, 1, 0]         # ba=8 group first
TAPS = [(ky - 1) * WP + (kx - 1) for ky in range(3) for kx in range(3)]

# which superchunks become available after which image is quantized:
SC_NEEDS_IMG = []
for _s in range(NSC):
    _hi = min(VSTART + 1536 * (_s + 1) + 59, APIX)
    SC_NEEDS_IMG.append(min((_hi - 1) // IMG, BS - 1))

# image i's last valid pixel lives in superchunk:
IMG_LAST_SC = {}
for _i in range(BS):
    _last = _i * IMG + H * WP + W
    _s = min((_last - VSTART) // SLEN, NSC - 1)
    IMG_LAST_SC[_s] = _i

_CACHE = {}


def _chunks_of_sc(s):
    out = []
    for c in range(SCCH):
        ci = s * SCCH + c
        if ci >= NCHUNK:
            break
        gs = VSTART + ci * CHUNK
        ln = min(CHUNK, VEND - gs)
        out.append((c * CHUNK, gs, ln))
    return out


def _build():
    nc = bacc.Bacc("TRN2", target_bir_lowering=False, debug=False,
                   num_devices=N_CORES)

    x_in = nc.dram_tensor("x", [BS, C, H, W], dt.float32, kind="ExternalInput")
    w1_in = nc.dram_tensor("conv1_w", [9, C, C, 3, 3], dt.float32, kind="ExternalInput")
    w2_in = nc.dram_tensor("conv2_w", [9, C, C, 3, 3], dt.float32, kind="ExternalInput")
    g1_in = nc.dram_tensor("gamma1", [C], dt.float32, kind="ExternalInput")
    b1_in = nc.dram_tensor("beta1", [C], dt.float32, kind="ExternalInput")
    g2_in = nc.dram_tensor("gamma2", [C], dt.float32, kind="ExternalInput")
    b2_in = nc.dram_tensor("beta2", [C], dt.float32, kind="ExternalInput")
    p1_in = nc.dram_tensor("p1", [9], dt.float32, kind="ExternalInput")
    p2_in = nc.dram_tensor("p2", [9], dt.float32, kind="ExternalInput")
    gn1_in = nc.dram_tensor("gn1", [9], dt.float32, kind="ExternalInput")
    gn2_in = nc.dram_tensor("gn2", [9], dt.float32, kind="ExternalInput")
    tau_in = nc.dram_tensor("tau", [1], dt.float32, kind="ExternalInput")
    consts_in = nc.dram_tensor("consts", [1, 27], dt.float32, kind="ExternalInput")
    out_dram = nc.dram_tensor("out", [BS, C, H, W], dt.float32, kind="ExternalOutput")

    from concourse.masks import make_identity

    with tile.TileContext(nc) as tc:
        with tc.tile_pool(name="main", bufs=1) as sb, \
             tc.tile_pool(name="ps", bufs=1, space="PSUM") as ps, \
             tc.tile_pool(name="dram", bufs=1, space="DRAM") as dram:

            # ---------- static tiles / input DMAs ----------
            ident = sb.tile([128, 128], dt.float32)
            make_identity(nc, ident[:])

            # memset on DVE, NOT GpSimd: the first gpsimd-queue instruction
            # gates the collectives firmware's one-time ~38us bootstrap
            # barrier, which in turn gates the BN1 AllReduce start.
            A8 = sb.tile([C, APIX], dt.float16, tag="A8")
            nc.vector.memset(A8[:], 0.0)  # zero borders once; writes stay interior

            wsrc1 = w1_in.ap().rearrange("k o i a b -> k o (i a b)")
            wsrc2 = w2_in.ap().rearrange("k o i a b -> k o (i a b)")

            def wdma(wsrc, ks, tag, dep=None):
                raws = {}
                for k in ks:
                    raw = sb.tile([C, 1152], dt.float32, tag="wraw", bufs=6,
                                  name=f"raw_{tag}_{k}")
                    d = nc.sync.dma_start(raw[:], wsrc[k])
                    if dep is not None:
                        add_dep_helper(d.ins, dep, reason="weight DMA yields to x")
                    raws[k] = raw
                return raws

            # x goes out first -- the BN1 AllReduce (gated on all of x) is the
            # startup critical path; weights have ~20us of slack behind it.
            x_sb = sb.tile([C, BS * NPIX_IMG], dt.float32, tag="big", name="x_sb")
            x_src = x_in.ap().rearrange("b c h w -> c b (h w)")
            x_dmas = []
            for i in range(BS):
                for hh in range(2):  # halves spread wider across DMA queues
                    lo_ = (i * 2 + hh) * (NPIX_IMG // 2)
                    x_dmas.append(nc.sync.dma_start(
                        x_sb[:, lo_:lo_ + NPIX_IMG // 2],
                        x_src[:, i, hh * (NPIX_IMG // 2):(hh + 1) * (NPIX_IMG // 2)]))

            raws1 = wdma(wsrc1, GROUP_KS[2], "w1")

            def row(name, t, n):
                r = sb.tile([1, n], dt.float32, name=name)
                nc.sync.dma_start(r[:], t.ap()[None, :])
                return r

            p1r = row("p1r", p1_in, 9)
            gn1r = row("gn1r", gn1_in, 9)
            p2r = row("p2r", p2_in, 9)
            gn2r = row("gn2r", gn2_in, 9)
            taur = row("taur", tau_in, 1)
            constsr = sb.tile([1, 27], dt.float32)
            nc.sync.dma_start(constsr[:], consts_in.ap())

            def col128(name, t):
                r = sb.tile([C, 1], dt.float32, name=name)
                nc.sync.dma_start(r[:], t.ap()[:, None])
                return r

            gam1, bet1 = col128("gam1", g1_in), col128("bet1", b1_in)
            gam2, bet2 = col128("gam2", g2_in), col128("bet2", b2_in)

            rtau = sb.tile([1, 1], dt.float32)
            nc.vector.reciprocal(rtau[:], taur[:])

            # ---------- per-layer softmax -> alpha/gamma strip -> broadcast ----------
            def softmax_strip(pr, gnr, tag):
                u = sb.tile([1, 9], dt.float32, name=f"u_{tag}")
                nc.vector.tensor_tensor(u[:], pr[:], gnr[:], Alu.add)
                nc.vector.tensor_scalar(u[:], u[:], rtau[:, 0:1], None, Alu.mult)
                mx = sb.tile([1, 1], dt.float32, name=f"mx_{tag}")
                nc.vector.tensor_reduce(mx[:], u[:], axis=mybir.AxisListType.X,
                                        op=Alu.max)
                nmx = sb.tile([1, 1], dt.float32, name=f"nmx_{tag}")
                nc.vector.tensor_scalar(nmx[:], mx[:], -1.0, None, Alu.mult)
                e = sb.tile([1, 9], dt.float32, name=f"e_{tag}")
                nc.scalar.activation(e[:], u[:], Act.Exp, bias=nmx[:, 0:1], scale=1.0)
                ssum = sb.tile([1, 1], dt.float32, name=f"ss_{tag}")
                nc.vector.tensor_reduce(ssum[:], e[:], axis=mybir.AxisListType.X,
                                        op=Alu.add)
                rsum = sb.tile([1, 1], dt.float32, name=f"rs_{tag}")
                nc.vector.reciprocal(rsum[:], ssum[:])
                wrow = sb.tile([1, 9], dt.float32, name=f"w_{tag}")
                nc.vector.tensor_scalar(wrow[:], e[:], rsum[:, 0:1], None, Alu.mult)
                strip = sb.tile([1, 12], dt.float32, name=f"strip_{tag}")
                nc.vector.tensor_tensor(strip[:, 0:9], wrow[:], constsr[:, 0:9],
                                        Alu.mult)
                pe1 = sb.tile([1, 9], dt.float32, name=f"pe1_{tag}")
                nc.vector.tensor_tensor(pe1[:], wrow[:], constsr[:, 9:18], Alu.mult)
                pe13 = pe1[:].rearrange("p (i g) -> p i g", g=3)
                for g in range(3):
                    nc.vector.tensor_reduce(strip[:, 9 + g:10 + g], pe13[:, :, g],
                                            axis=mybir.AxisListType.X, op=Alu.add,
                                            negate=True)
                bcast = sb.tile([C, 12], dt.float32, name=f"bcast_{tag}")
                nc.gpsimd.partition_broadcast(bcast[:], strip[:])
                return bcast

            # ---------- BN batch stats: sum on DVE, sum-of-squares on ACT ----
            def img_stats_flat(src2d, ssum, ssq, col, tag):
                nc.vector.tensor_reduce(ssum[:, col:col + 1], src2d,
                                        axis=mybir.AxisListType.X, op=Alu.add)
                n = src2d.shape[1]
                scr2 = sb.tile([C, NPIX_IMG], dt.float32, tag="scr", bufs=2,
                               name=f"scq_{tag}_{col}")
                nc.scalar.activation(scr2[:, 0:n], src2d, Act.Square, bias=0.0,
                                     scale=1.0, accum_out=ssq[:, col:col + 1])

            def img_stats_3d(src3d, ssum, ssq, col, rowscr, tag):
                # strided source: DVE reduces innermost dim then rows
                a = src3d.shape[1]
                nc.vector.tensor_reduce(rowscr[:, 0:a], src3d,
                                        axis=mybir.AxisListType.X, op=Alu.add)
                nc.vector.tensor_reduce(ssum[:, col:col + 1], rowscr[:, 0:a],
                                        axis=mybir.AxisListType.X, op=Alu.add)
                n = a * src3d.shape[2]
                scr2 = sb.tile([C, NPIX_IMG], dt.float32, tag="scr", bufs=2,
                               name=f"scq_{tag}_{col}")
                scr23 = scr2[:, 0:n].rearrange("p (a b) -> p a b",
                                               b=src3d.shape[2])
                nc.scalar.activation(scr23, src3d, Act.Square, bias=0.0,
                                     scale=1.0, accum_out=ssq[:, col:col + 1])

            def ar_local(ssum, ssq, tag):
                loc = sb.tile([C, 2], dt.float32, name=f"loc_{tag}")
                nc.vector.tensor_reduce(loc[:, 0:1], ssum[:],
                                        axis=mybir.AxisListType.X, op=Alu.add)
                nc.vector.tensor_reduce(loc[:, 1:2], ssq[:],
                                        axis=mybir.AxisListType.X, op=Alu.add)
                cin = dram.tile([C, 2], dt.float32, name=f"ccin_{tag}")
                nc.sync.dma_start(cin[:], loc[:])
                return cin

            def ar_go(cin, tag):
                cout = dram.tile([C, 2], dt.float32, addr_space="Shared",
                                 name=f"ccout_{tag}")
                nc.gpsimd.collective_compute(
                    "AllReduce", Alu.add,
                    replica_groups=[list(range(N_CORES))],
                    ins=[cin.opt()], outs=[cout.opt()])
                glob = sb.tile([C, 2], dt.float32, name=f"glob_{tag}")
                nc.sync.dma_start(glob[:], cout[:])
                return glob

            def bn_scalars(glob, gam, bet, tag):
                def t1(name):
                    return sb.tile([C, 1], dt.float32, name=f"{name}_{tag}")
                mean, e2, msq, var, ve = (t1("mean"), t1("e2"), t1("msq"),
                                          t1("var"), t1("ve"))
                nc.vector.tensor_scalar(mean[:], glob[:, 0:1], 1.0 / NTOT, None,
                                        Alu.mult)
                nc.vector.tensor_scalar(e2[:], glob[:, 1:2], 1.0 / NTOT, None,
                                        Alu.mult)
                nc.vector.tensor_tensor(msq[:], mean[:], mean[:], Alu.mult)
                nc.vector.tensor_tensor(var[:], e2[:], msq[:], Alu.subtract)
                nc.vector.tensor_scalar(ve[:], var[:], EPS, None, Alu.add)
                sq, y = t1("sq"), t1("y0")
                nc.scalar.activation(sq[:], ve[:], Act.Sqrt, bias=0.0, scale=1.0)
                nc.vector.reciprocal(y[:], sq[:])
                for it in range(2):  # Newton: y <- y*(1.5 - 0.5*ve*y^2)
                    tt1, tt2, tt3, yn = (t1(f"n{it}a"), t1(f"n{it}b"),
                                         t1(f"n{it}c"), t1(f"y{it + 1}"))
                    nc.vector.tensor_tensor(tt1[:], y[:], y[:], Alu.mult)
                    nc.vector.tensor_tensor(tt2[:], tt1[:], ve[:], Alu.mult)
                    nc.vector.tensor_scalar(tt3[:], tt2[:], -0.5, 1.5, Alu.mult,
                                            Alu.add)
                    nc.vector.tensor_tensor(yn[:], y[:], tt3[:], Alu.mult)
                    y = yn
                sbn, bt, sq_, bq_ = (t1("sbn"), t1("bt"), t1("sclq"), t1("biasq"))
                nc.vector.tensor_tensor(sbn[:], gam[:], y[:], Alu.mult)
                nc.vector.tensor_tensor(bt[:], mean[:], sbn[:], Alu.mult)
                # u = 255*(s*x + b): sq = 255*s ; bq = 255*(beta - mean*s)
                nc.vector.tensor_scalar(bq_[:], bt[:], -255.0, bet255(bet, tag),
                                        Alu.mult, Alu.add)
                nc.vector.tensor_scalar(sq_[:], sbn[:], 255.0, None, Alu.mult)
                return sq_, bq_

            _bet255 = {}

            def bet255(bet, tag):
                if tag not in _bet255:
                    b = sb.tile([C, 1], dt.float32, name=f"bet255_{tag}")
                    nc.vector.tensor_scalar(b[:], bet[:], 255.0, None, Alu.mult)
                    _bet255[tag] = b
                return _bet255[tag][:, 0:1]

            # ---------- quantize one image into the 8-bit grid ----------
            def quantize_img(src3d, A8t, sq_, bq_, i, tag, r0=0, r1=H):
                # u = relu(255*(s*x+b)); round+clamp: (min(u,255)+M)-M
                nr = r1 - r0
                u = sb.tile([C, NPIX_IMG], dt.float32, tag="scr", bufs=2,
                            name=f"qu_{tag}_{i}_{r0}")
                u3 = u[:, 0:nr * W].rearrange("p (a b) -> p a b", a=nr)
                nc.scalar.activation(u3, src3d[:, r0:r1], Act.Relu,
                                     bias=bq_[:, 0:1], scale=sq_[:, 0:1])
                nc.vector.tensor_scalar(u[:, 0:nr * W], u[:, 0:nr * W], 255.0,
                                        MAGIC, Alu.min, Alu.add)
                dst = A8t[:, i * IMG + BASE + r0 * WP:
                          i * IMG + BASE + r1 * WP]
                dst3 = dst.rearrange("p (a b) -> p a b", b=WP)[:, :, 0:W]
                nc.vector.tensor_scalar(dst3, u3, MAGIC, None, Alu.subtract)

            # ---------- weight preparation ----------
            def group_r2(raws, ks, gi, tag):
                """abs-max per candidate -> r2 = 1/(2 tanh(amax)) column [C,3].

                abs-max per partition on DVE (the GpSimd cross-lane reduce
                ignores apply_absolute_value on hardware), then the partition
                max of the nonnegative column on GpSimd -- no transpose."""
                amaxc = sb.tile([C, 3], dt.float32, name=f"amaxc_{tag}_{gi}")
                mrow = sb.tile([1, 3], dt.float32, name=f"mrow_{tag}_{gi}")
                for j, k in enumerate(ks):
                    nc.vector.tensor_reduce(amaxc[:, j:j + 1], raws[k][:],
                                            axis=mybir.AxisListType.X,
                                            op=Alu.max,
                                            apply_absolute_value=True)
                    nc.gpsimd.tensor_reduce(mrow[0:1, j:j + 1],
                                            amaxc[:, j:j + 1],
                                            axis=mybir.AxisListType.C,
                                            op=Alu.max)
                tam = sb.tile([1, 3], dt.float32, name=f"tam_{tag}_{gi}")
                nc.scalar.activation(tam[:], mrow[:], Act.Tanh, bias=0.0, scale=1.0)
                a2 = sb.tile([1, 3], dt.float32, name=f"a2_{tag}_{gi}")
                nc.vector.tensor_scalar(a2[:], tam[:], 2.0, None, Alu.mult)
                r2r = sb.tile([1, 3], dt.float32, name=f"r2r_{tag}_{gi}")
                nc.vector.reciprocal(r2r[:], a2[:])
                r2g = sb.tile([C, 3], dt.float32, name=f"r2g_{tag}_{gi}")
                nc.gpsimd.partition_broadcast(r2g[:], r2r[:])
                return r2g

            def prep_chain(raws, ks, r2g, bcast, tag, g):
                """accumulate one ba-group's quantized candidates -> wacc."""
                wacc = None
                for j, k in enumerate(ks):
                    th = sb.tile([C, 1152], dt.float32, tag="wth", bufs=2,
                                 name=f"th_{tag}_{k}")
                    nc.scalar.activation(th[:], raws[k][:], Act.Tanh, bias=0.0,
                                         scale=1.0)
                    # wn = th/(2amax)+0.5 ; u2 = wn*nw + M (rounds) ; m = u2-M
                    # (+0.5*nw must NOT fold into M: M+127.5 isn't fp32-exact)
                    nc.vector.tensor_scalar(th[:], th[:], r2g[:, j:j + 1], 0.5,
                                            Alu.mult, Alu.add)
                    nc.vector.tensor_scalar(th[:], th[:], float(NW[k]), MAGIC,
                                            Alu.mult, Alu.add)
                    nc.vector.tensor_scalar(th[:], th[:], MAGIC, None,
                                            Alu.subtract)
                    if j == 0:
                        wacc = sb.tile([C, 1152], dt.float32, tag="wacc",
                                       bufs=2, name=f"wacc_{tag}_{g}_{j}")
                        nc.vector.tensor_scalar(wacc[:], th[:], bcast[:, k:k + 1],
                                                bcast[:, 9 + g:10 + g],
                                                Alu.mult, Alu.add)
                    else:
                        nxt = sb.tile([C, 1152], dt.float32, tag="wacc",
                                      bufs=2, name=f"wacc_{tag}_{g}_{j}")
                        nc.vector.scalar_tensor_tensor(nxt[:], th[:],
                                                       bcast[:, k:k + 1], wacc[:],
                                                       Alu.mult, Alu.add)
                        wacc = nxt
                return wacc

            def prep_transpose(wacc, tag, g, Wt):
                w3 = wacc[:].rearrange("p (i t) -> p i t", t=9)
                for t in range(9):
                    tp = ps.tile([128, 128], dt.float32, tag="tps", bufs=2,
                                 name=f"tp_{tag}_{g}_{t}")
                    nc.tensor.transpose(tp[:], w3[:, :, t], ident[:])
                    nc.scalar.activation(Wt[g][:, t, :], tp[:], Act.Copy,
                                         bias=0.0, scale=WSCALE)

            def alloc_W(tag):
                return [sb.tile([C, 9, C], dt.float16, name=f"W_{tag}_{g}")
                        for g in range(3)]

            # ---------- conv pieces ----------
            def derive_sc(src8, s, tag):
                start = VSTART + s * SLEN
                lo = min(max((start - 64) & ~1, 0), APIX - STG)
                a4s = sb.tile([C, STG], dt.float16, tag="a4s", bufs=3,
                              name=f"a4_{tag}_{s}")
                nc.vector.tensor_scalar(a4s[:], src8[:, lo:lo + STG], 1.0 / 17.0,
                                        MAGICH, Alu.mult, Alu.add)
                nc.vector.tensor_scalar(a4s[:], a4s[:], MAGICH, None, Alu.subtract)
                a2s = sb.tile([C, STG], dt.float16, tag="a2s", bufs=3,
                              name=f"a2_{tag}_{s}")
                nc.vector.tensor_scalar(a2s[:], a4s[:], 1.0 / 5.0, MAGICH,
                                        Alu.mult, Alu.add)
                nc.vector.tensor_scalar(a2s[:], a2s[:], MAGICH, None, Alu.subtract)
                return a4s, a2s, lo

            def conv_sc(Wt, src8, a4s, a2s, lo, cdst, s, tag, groups=None):
                """emit conv passes for superchunk s; groups=None -> all."""
                chunks = _chunks_of_sc(s)
                start = VSTART + s * SLEN
                pt = _sc_psum(tag, s)
                for pi, g in enumerate(GORDER):
                    if groups is not None and g not in groups:
                        continue
                    for t in range(9):
                        off = TAPS[t]
                        for (pcol, gs, ln) in chunks:
                            if g == 2:
                                rhs = src8[:, gs + off:gs + off + ln]
                            elif g == 1:
                                rhs = a4s[:, gs + off - lo:gs + off - lo + ln]
                            else:
                                rhs = a2s[:, gs + off - lo:gs + off - lo + ln]
                            nc.tensor.matmul(
                                pt[:, pcol:pcol + ln], Wt[g][:, t, :], rhs,
                                start=(pi == 0 and t == 0),
                                stop=(pi == 2 and t == 8))
                if groups is None or 0 in groups:
                    sc_end = min(start + SLEN, VEND)
                    nc.scalar.activation(cdst[:, start:sc_end],
                                         pt[:, 0:sc_end - start], Act.Copy,
                                         bias=0.0, scale=1.0 / WSCALE)

            _psums = {}

            def _sc_psum(tag, s):
                key = (tag, s)
                if key not in _psums:
                    _psums[key] = ps.tile([128, SLEN], dt.float32, tag="cps",
                                          bufs=2, name=f"ps_{tag}_{s}")
                return _psums[key]

            # ================= LAYER 1 =================
            ssum1 = sb.tile([C, 5], dt.float32)
            ssq1 = sb.tile([C, 5], dt.float32)
            nc.vector.memset(ssum1[:], 0.0)
            nc.vector.memset(ssq1[:], 0.0)
            x3 = x_sb[:].rearrange("p (b a w) -> p b a w", b=BS, a=H)
            for i in range(BS):
                img_stats_flat(x_sb[:, i * NPIX_IMG:(i + 1) * NPIX_IMG],
                               ssum1, ssq1, i, "s1")
            cin1 = ar_local(ssum1, ssq1, "c1")

            # g1 candidates land in fresh slots; g0's DMAs are emitted only
            # after g2's chain reads (slot reuse: 6 bufs, 9 candidates)
            raws1.update(wdma(wsrc1, GROUP_KS[1], "w1"))

            r2g1 = {2: group_r2(raws1, GROUP_KS[2], 0, "w1")}

            glob1 = ar_go(cin1, "c1")
            bc1 = softmax_strip(p1r, gn1r, "l1")
            bc2 = softmax_strip(p2r, gn2r, "l2")

            W1 = alloc_W("w1")
            c1 = sb.tile([C, APIX], dt.float32, tag="big", name="c1buf")
            ssum2 = sb.tile([C, 5], dt.float32)
            ssq2 = sb.tile([C, 5], dt.float32)
            nc.vector.memset(ssum2[:], 0.0)
            nc.vector.memset(ssq2[:], 0.0)
            rowscr = sb.tile([C, H], dt.float32, name="rowscr")

            # group-2 weights finalize while the collectives bootstrap
            # barrier + AllReduce (~83us) are in flight
            wacc = prep_chain(raws1, GROUP_KS[2], r2g1[2], bc1, "w1", 2)
            prep_transpose(wacc, "w1", 2, W1)
            raws1.update(wdma(wsrc1, GROUP_KS[0], "w1"))

            nsq1, nbq1 = bn_scalars(glob1, gam1, bet1, "bn1")

            def c1_img3d(i, r0=0, r1=H):
                off = i * IMG + BASE + r0 * WP
                v = c1[:, off:off + (r1 - r0) * WP]
                return v.rearrange("p (a b) -> p a b", b=WP)[:, :, 0:W]

            # NOTE: all of x must be consumed (quantized) before conv1's first
            # PSUM copy writes c1 -- they share one SBUF slot and the slot
            # handover is tile-granular.  The first copy comes after group-0's
            # sc0 pass, so imgs 2/3 quantize between the g1 and g0 passes.
            # The g2 passes read only the A8 grid, so they start as soon as
            # img0 is quantized; derives and later-group weight chains overlap
            # the g2 matmul sweep.
            quantize_img(x3[:, 0], A8, nsq1, nbq1, 0, "q1")
            a4s0, a2s0, lo0 = derive_sc(A8[:], 0, "cv1")
            conv_sc(W1, A8[:], a4s0, a2s0, lo0, c1, 0, "cv1", groups=[2])
            quantize_img(x3[:, 1], A8, nsq1, nbq1, 1, "q1")
            a4s1, a2s1, lo1 = derive_sc(A8[:], 1, "cv1")
            conv_sc(W1, A8[:], a4s1, a2s1, lo1, c1, 1, "cv1", groups=[2])

            r2g1[1] = group_r2(raws1, GROUP_KS[1], 1, "w1")
            wacc = prep_chain(raws1, GROUP_KS[1], r2g1[1], bc1, "w1", 1)
            prep_transpose(wacc, "w1", 1, W1)
            conv_sc(W1, A8[:], a4s0, a2s0, lo0, c1, 0, "cv1", groups=[1])
            conv_sc(W1, A8[:], a4s1, a2s1, lo1, c1, 1, "cv1", groups=[1])

            quantize_img(x3[:, 2], A8, nsq1, nbq1, 2, "q1")
            quantize_img(x3[:, 3], A8, nsq1, nbq1, 3, "q1")

            r2g1[0] = group_r2(raws1, GROUP_KS[0], 2, "w1")
            wacc = prep_chain(raws1, GROUP_KS[0], r2g1[0], bc1, "w1", 0)
            prep_transpose(wacc, "w1", 0, W1)
            conv_sc(W1, A8[:], a4s0, a2s0, lo0, c1, 0, "cv1", groups=[0])
            conv_sc(W1, A8[:], a4s1, a2s1, lo1, c1, 1, "cv1", groups=[0])

            def cv1_after_sc(s):
                if s == 7:  # partial img3 stats (rows 1..37 available)
                    img_stats_3d(c1_img3d(3, 0, 37), ssum2, ssq2, 3, rowscr, "s2")
                if s in IMG_LAST_SC:
                    i = IMG_LAST_SC[s]
                    if i == 3:
                        img_stats_3d(c1_img3d(3, 37, H), ssum2, ssq2, 4, rowscr,
                                     "s2")
                    else:
                        img_stats_3d(c1_img3d(i), ssum2, ssq2, i, rowscr, "s2")

            cv1_after_sc(0), cv1_after_sc(1)
            r2g2 = {}
            for s in range(2, NSC):
                a4s, a2s, lo = derive_sc(A8[:], s, "cv1")
                conv_sc(W1, A8[:], a4s, a2s, lo, c1, s, "cv1")
                cv1_after_sc(s)
                # layer-2 weight prep spread across conv1 superchunks so each
                # group's PE transposes are ready when the FIFO reaches them
                if s == 2:
                    raws2 = wdma(wsrc2, GROUP_KS[2] + GROUP_KS[1], "w2")
                    W2 = alloc_W("w2")
                elif s in (3, 4, 5):
                    g = GORDER[s - 3]
                    r2g2[g] = group_r2(raws2, GROUP_KS[g], s - 3, "w2")
                    wacc_g = prep_chain(raws2, GROUP_KS[g], r2g2[g], bc2,
                                        "w2", g)
                    prep_transpose(wacc_g, "w2", g, W2)
                    if s == 4:
                        # g0 slots reuse g2's: DMA only after g2's chain reads
                        raws2.update(wdma(wsrc2, GROUP_KS[0], "w2"))

            # ================= LAYER 2 =================
            cin2 = ar_local(ssum2, ssq2, "c2")
            glob2 = ar_go(cin2, "c2")
            nsq2, nbq2 = bn_scalars(glob2, gam2, bet2, "bn2")

            # layer-2 grid shares the A8 slot: zero borders are bit-identical
            A8h = sb.tile([C, APIX], dt.float16, tag="A8", name="A8h")

            out_v = out_dram.ap().rearrange("b c h w -> c b (h w)")

            def residual_out(i, r0=0, r1=H, pre=None):
                n = (r1 - r0) * W
                if pre is None:
                    xr = sb.tile([C, NPIX_IMG], dt.float32, tag="scr", bufs=2,
                                 name=f"xr_{i}_{r0}")
                    nc.sync.dma_start(xr[:, 0:n], x_src[:, i, r0 * W:r1 * W])
                    xrv = xr[:, 0:n]
                else:
                    xrv = pre[:]
                xr3 = xrv.rearrange("p (a b) -> p a b", a=r1 - r0)
                nc.vector.tensor_tensor(xr3, xr3, c1_img3d(i, r0, r1), Alu.add)
                nc.sync.dma_start(out_v[:, i, r0 * W:r1 * W], xrv)

            # img3's residual x is prefetched into dedicated tiles so the
            # final superchunk's add + store isn't DMA-latency-bound
            xr3a = sb.tile([C, 37 * W], dt.float32, name="xr3a")
            xr3b = sb.tile([C, (H - 37) * W], dt.float32, name="xr3b")

            _q2_done = {}
            for s in range(0, NSC):
                need = SC_NEEDS_IMG[s]
                for i in range(BS):
                    if need >= i and not _q2_done.get(i):
                        quantize_img(c1_img3d(i), A8h, nsq2, nbq2, i, "q2")
                        _q2_done[i] = True
                a4s, a2s, lo = derive_sc(A8h[:], s, "cv2")
                conv_sc(W2, A8h[:], a4s, a2s, lo, c1, s, "cv2")
                if s == 5:
                    nc.sync.dma_start(xr3a[:], x_src[:, 3, 0:37 * W])
                    nc.sync.dma_start(xr3b[:], x_src[:, 3, 37 * W:H * W])
                if s == 7:
                    residual_out(3, 0, 37, pre=xr3a)
                if s in IMG_LAST_SC:
                    i = IMG_LAST_SC[s]
                    if i == 3:
                        residual_out(3, 37, H, pre=xr3b)
                    else:
                        residual_out(i)

    nc.compile()
    return nc


def _consts():
    c = np.zeros((1, 27), np.float32)
    for k in range(9):
        c[0, k] = 2.0 / (NW[k] * NA[k])
        c[0, 9 + k] = 1.0 / NA[k]
        c[0, 18 + k] = float(NW[k])
    return c


def _in_maps(inputs):
    x = np.ascontiguousarray(inputs["x"], dtype=np.float32)
    shared = {
        "conv1_w": np.ascontiguousarray(inputs["conv1_w"], dtype=np.float32),
        "conv2_w": np.ascontiguousarray(inputs["conv2_w"], dtype=np.float32),
        "gamma1": np.ascontiguousarray(inputs["gamma1"], dtype=np.float32),
        "beta1": np.ascontiguousarray(inputs["beta1"], dtype=np.float32),
        "gamma2": np.ascontiguousarray(inputs["gamma2"], dtype=np.float32),
        "beta2": np.ascontiguousarray(inputs["beta2"], dtype=np.float32),
        "p1": np.ascontiguousarray(inputs["p1"], dtype=np.float32),
        "p2": np.ascontiguousarray(inputs["p2"], dtype=np.float32),
        "gn1": np.ascontiguousarray(inputs["gn1"], dtype=np.float32),
        "gn2": np.ascontiguousarray(inputs["gn2"], dtype=np.float32),
        "tau": np.asarray(inputs["tau"], dtype=np.float32).reshape(1),
        "consts": _consts(),
    }
    return [dict(shared, x=x[c * BS:(c + 1) * BS]) for c in range(N_CORES)]


def _get_nc():
    if "nc" not in _CACHE:
        _CACHE["nc"] = _build()
    return _CACHE["nc"]


def _run(in_maps, trace=False):
    nc = _get_nc()
    return bass_utils.run_bass_kernel_spmd(
        nc, in_maps, core_ids=list(range(N_CORES)), trace=trace)


def kernel(**inputs) -> np.ndarray:
    res = _run(_in_maps(inputs))
    return np.concatenate([res.results[c]["out"] for c in range(N_CORES)], axis=0)
